# revision 9
# baseline (speedup 1.0000x reference)
"""Trainium2 Bass kernel for nn_Block_523986010339 (PVT-style transformer block).

Sharding: data-parallel over batch B=8 -> one batch element per NeuronCore.
Per-core layouts:
  - residual stream token-major fp32 [128p=token%128, 128t=token//128, 64c]
  - matmul operands channel-major bf16 [c, n], n = 128*y + x
  - LN mean folded into matmul weights via an extra "m*g" row; rsqrt scale
    applied token-major with broadcast APs
  - attention: S^T channel-major, exp without max-subtraction (tiny logits),
    denominator via fused ones-column in the V matmul, divided out after proj
  - MLP: fc1 and 3x3 depthwise conv fused into 9 accumulated matmuls over a
    zero-guarded channel-major layout (row pitch 130)
"""

import functools
import json

import numpy as np
import ml_dtypes

import concourse.bass as bass
import concourse.mybir as mybir
import concourse.tile as tile
from concourse.bass_utils import run_bass_kernel_spmd
from concourse.masks import make_identity

F32 = mybir.dt.float32
BF16 = mybir.dt.bfloat16
BF = ml_dtypes.bfloat16

B, N, C, H, W = 8, 16384, 64, 128, 128
SR, HID, NR = 8, 256, 256
P, T = 128, 128
RP = W + 2          # guarded row pitch
PAD = RP + 1        # head/tail pad so all tap offsets stay in-bounds
NG = PAD + RP * (H + 2) + PAD
AX = mybir.AxisListType
OP = mybir.AluOpType
AF = mybir.ActivationFunctionType


def _split_excess_waits(nc, max_waits=1):
    """walrus in this container rejects >1 sync wait per instruction; move
    excess waits onto injected Drain instructions just before the owner."""
    d = json.loads(mybir.module_to_json_string(nc.m))
    n_split = [0]

    def fix(insts):
        out = []
        for inst in insts:
            si = inst.get("sync_info") or {}
            waits = si.get("on_wait") or []
            if len(waits) > max_waits:
                extra = waits[:-max_waits]
                for i in range(0, len(extra), max_waits):
                    n_split[0] += 1
                    out.append({
                        "name": f"WSPLIT-{n_split[0]}",
                        "opcode": "Drain",
                        "engine": inst["engine"],
                        "ins": [],
                        "outs": [],
                        "is_reset_sema": False,
                        "sync_info": {"on_update": [],
                                      "on_wait": extra[i:i + max_waits]},
                    })
                si["on_wait"] = waits[-max_waits:]
                inst["sync_info"] = si
            out.append(inst)
        return out

    for f in d.get("functions", []):
        for bb in f.get("blocks", []):
            bb["instructions"] = fix(bb["instructions"])
    nc.m = mybir.module_from_json_string(json.dumps(d))


def _ln_stats(nc, sc, x_tm, xsq_view, sq_scr, epst, nt):
    """Token-major LN stats: returns (g, mg) tiles [128, nt] fp32 given
    x_tm [128, nt, 64] fp32, a bf16 scratch for x^2 and its [128,nt,64] view."""
    nc.scalar.square(out=sq_scr, in_=x_tm.rearrange("p t c -> p (t c)"))
    s1 = sc.tile([P, nt], F32, tag=f"s1_{nt}")
    s2 = sc.tile([P, nt], F32, tag=f"s2_{nt}")
    nc.vector.tensor_reduce(out=s1, in_=x_tm, axis=AX.X, op=OP.add)
    nc.vector.tensor_reduce(out=s2, in_=xsq_view, axis=AX.X, op=OP.add)
    mean = sc.tile([P, nt], F32, tag=f"mean_{nt}")
    var = sc.tile([P, nt], F32, tag=f"var_{nt}")
    nc.vector.tensor_scalar_mul(out=mean, in0=s1, scalar1=1.0 / C)
    nc.vector.tensor_scalar_mul(out=var, in0=s2, scalar1=1.0 / C)
    mm = sc.tile([P, nt], F32, tag=f"mm_{nt}")
    nc.vector.tensor_tensor(out=mm, in0=mean, in1=mean, op=OP.mult)
    nc.vector.tensor_tensor(out=var, in0=var, in1=mm, op=OP.subtract)
    sd = sc.tile([P, nt], F32, tag=f"sd_{nt}")
    nc.scalar.activation(out=sd, in_=var, func=AF.Sqrt, bias=epst, scale=1.0)
    g = sc.tile([P, nt], F32, tag=f"g_{nt}")
    nc.vector.reciprocal(out=g, in_=sd)
    mg = sc.tile([P, nt], F32, tag=f"mg_{nt}")
    nc.vector.tensor_tensor(out=mg, in0=mean, in1=g, op=OP.mult)
    return g, mg


def _build_nc():
    nc = bass.Bass("TRN2")
    x_d = nc.dram_tensor("x", [N, C], F32, kind="ExternalInput")
    out_d = nc.dram_tensor("out", [N, C], F32, kind="ExternalOutput")
    wq_d = nc.dram_tensor("wq", [C + 1, C], BF16, kind="ExternalInput")
    bq_d = nc.dram_tensor("bq", [C, 1], F32, kind="ExternalInput")
    wsr_d = nc.dram_tensor("wsr", [64, C + 1, C], BF16, kind="ExternalInput")
    bsr_d = nc.dram_tensor("bsr", [C, 1], F32, kind="ExternalInput")
    wkv_d = nc.dram_tensor("wkv", [C, 2 * C], BF16, kind="ExternalInput")
    bkv_d = nc.dram_tensor("bkv", [2 * C, 1], F32, kind="ExternalInput")
    wpj_d = nc.dram_tensor("wpj", [C + 1, C + 1], BF16, kind="ExternalInput")
    wm_d = nc.dram_tensor("wm", [18, C + 1, 128], BF16, kind="ExternalInput")
    bg_d = nc.dram_tensor("bg", [128, 2], F32, kind="ExternalInput")
    wf2_d = nc.dram_tensor("wf2", [2, 128, C], BF16, kind="ExternalInput")
    bf2_d = nc.dram_tensor("bf2", [C, 1], F32, kind="ExternalInput")

    with tile.TileContext(nc) as tc:
        with (
            tc.tile_pool(name="consts", bufs=1) as consts,
            tc.tile_pool(name="big", bufs=1) as big,
            tc.tile_pool(name="sc", bufs=2) as sc,
            tc.tile_pool(name="ch", bufs=3) as ch,
            tc.tile_pool(name="psA", bufs=4, space="PSUM") as psA,
            tc.tile_pool(name="psT", bufs=2, space="PSUM") as psT,
        ):
            ident = consts.tile([128, 128], BF16)
            make_identity(nc, ident)
            wq = consts.tile([C + 1, C], BF16)
            nc.gpsimd.dma_start(out=wq, in_=wq_d[:, :])
            wsr = consts.tile([C + 1, 64, C], BF16)
            nc.gpsimd.dma_start(out=wsr, in_=wsr_d.rearrange("k c o -> c k o"))
            wkv = consts.tile([C, 2 * C], BF16)
            nc.gpsimd.dma_start(out=wkv, in_=wkv_d[:, :])
            wpj = consts.tile([C + 1, C + 1], BF16)
            nc.gpsimd.dma_start(out=wpj, in_=wpj_d[:, :])
            wm = consts.tile([C + 1, 18, 128], BF16)
            nc.gpsimd.dma_start(out=wm, in_=wm_d.rearrange("t c m -> c t m"))
            wf2 = consts.tile([128, 2, C], BF16)
            nc.gpsimd.dma_start(out=wf2, in_=wf2_d.rearrange("g k m -> k g m"))
            bq = consts.tile([C, 1], F32)
            nc.gpsimd.dma_start(out=bq, in_=bq_d[:, :])
            bsr = consts.tile([C, 1], F32)
            nc.gpsimd.dma_start(out=bsr, in_=bsr_d[:, :])
            bkv = consts.tile([2 * C, 1], F32)
            nc.gpsimd.dma_start(out=bkv, in_=bkv_d[:, :])
            bg = consts.tile([128, 2], F32)
            nc.gpsimd.dma_start(out=bg, in_=bg_d[:, :])
            bf2 = consts.tile([C, 1], F32)
            nc.gpsimd.dma_start(out=bf2, in_=bf2_d[:, :])
            epst = consts.tile([P, 1], F32)
            nc.vector.memset(epst, 1e-5)

            # ---- load x (token-major) ----
            x_tm = big.tile([P, T, C], F32, tag="xr")
            x_v = x_d.rearrange("(t p) c -> p t c", p=P)
            for q4 in range(4):
                nc.sync.dma_start(out=x_tm[:, 32 * q4:32 * (q4 + 1), :],
                                  in_=x_v[:, 32 * q4:32 * (q4 + 1), :])

            # ---- LN1 ----
            sq_scr = big.tile([P, T * C], BF16, tag="scr")
            g1, mg1 = _ln_stats(nc, sc, x_tm,
                                sq_scr.rearrange("p (t c) -> p t c", c=C),
                                sq_scr, epst, T)
            a1tm = big.tile([P, T, C + 1], BF16, tag="scr2")
            nc.vector.tensor_tensor(
                out=a1tm[:, :, 0:C], in0=x_tm,
                in1=g1[:, :, None].broadcast_to([P, T, C]), op=OP.mult)
            nc.vector.tensor_copy(out=a1tm[:, :, C], in_=mg1)

            # transpose A1 to channel-major [65, N]
            a1cm = big.tile([C + 1, N], BF16, tag="acm")
            a1cm_v = a1cm.rearrange("c (j k n) -> c j k n", j=32, k=4)
            for j in range(32):
                pt = psT.tile([C + 1, 4, 128], BF16, tag="tp")
                for k in range(4):
                    nc.tensor.transpose(out=pt[:, k, :],
                                        in_=a1tm[:, 4 * j + k, :],
                                        identity=ident)
                nc.scalar.copy(out=a1cm_v[:, j, :, :], in_=pt)

            # ---- Q^T = wq @ A1 ----
            qt = big.tile([C, N], BF16, tag="qt")
            for i in range(32):
                ps = psA.tile([128, 512], F32, tag="ps", name="ps")[0:C, :]
                nc.tensor.matmul(out=ps, lhsT=wq,
                                 rhs=a1cm[:, 512 * i:512 * (i + 1)],
                                 start=True, stop=True)
                nc.scalar.activation(out=qt[:, 512 * i:512 * (i + 1)], in_=ps,
                                     func=AF.Identity, bias=bq, scale=1.0)

            # ---- spatial reduction conv (8x8 stride 8) ----
            a1sr = a1cm.rearrange("c (Y ky X kx) -> c ky kx Y X", ky=SR, kx=SR, X=16)
            psr = psA.tile([128, 512], F32, tag="ps", name="ps").rearrange("c (a y x) -> c a y x", a=2, y=16)[0:C, 0, :, :]
            for kk in range(64):
                ky, kx = kk // 8, kk % 8
                nc.tensor.matmul(out=psr, lhsT=wsr[:, kk, :],
                                 rhs=a1sr[:, ky, kx, :, :],
                                 start=(kk == 0), stop=(kk == 63))
            xrcm = consts.tile([C, NR], BF16)
            nc.scalar.activation(out=xrcm.rearrange("c (y x) -> c y x", x=16),
                                 in_=psr, func=AF.Identity,
                                 bias=bsr, scale=1.0)

            # ---- LN on reduced tokens (srn), token-major ----
            xr_tm = consts.tile([P, 2, C], F32)
            for hh in range(2):
                pv = psT.tile([128, C], BF16, tag="tp")
                nc.tensor.transpose(out=pv, in_=xrcm[:, 128 * hh:128 * (hh + 1)],
                                    identity=ident[0:C, 0:C])
                nc.vector.tensor_copy(out=xr_tm[:, hh, :], in_=pv)
            sq2 = consts.tile([P, 2, C], BF16)
            g_r, mg_r = _ln_stats(nc, sc, xr_tm,
                                  sq2, sq2.rearrange("p t c -> p (t c)"),
                                  epst, 2)
            ar_tm = consts.tile([P, 2, C], BF16)
            nc.vector.tensor_tensor(
                out=ar_tm, in0=xr_tm,
                in1=g_r[:, :, None].broadcast_to([P, 2, C]), op=OP.mult)
            mgb = sc.tile([P, 2, C], BF16, tag="mgb")
            nc.vector.tensor_tensor(
                out=mgb, in0=mg_r[:, :, None].broadcast_to([P, 2, C]),
                in1=g_r[:, :, None].broadcast_to([P, 2, C]), op=OP.bypass)
            nc.vector.tensor_tensor(out=ar_tm, in0=ar_tm, in1=mgb, op=OP.subtract)
            arcm = consts.tile([C, NR], BF16)
            for hh in range(2):
                pv = psT.tile([C, 128], BF16, tag="tp")
                nc.tensor.transpose(out=pv, in_=ar_tm[:, hh, :], identity=ident)
                nc.vector.tensor_copy(out=arcm[:, 128 * hh:128 * (hh + 1)], in_=pv)

            # ---- KV ----
            pkv = psA.tile([128, 512], F32, tag="ps", name="ps")[:, 0:NR]
            nc.tensor.matmul(out=pkv, lhsT=wkv, rhs=arcm, start=True, stop=True)
            kvcm = consts.tile([2 * C, NR], BF16)
            nc.scalar.activation(out=kvcm, in_=pkv, func=AF.Identity,
                                 bias=bkv, scale=1.0)
            vp = consts.tile([128, 2, C + 1], BF16)
            nc.vector.memset(vp[:, :, C:C + 1], 1.0)
            for hh in range(2):
                pv = psT.tile([128, C], BF16, tag="tp")
                nc.tensor.transpose(out=pv,
                                    in_=kvcm[C:2 * C, 128 * hh:128 * (hh + 1)],
                                    identity=ident[C:2 * C, C:2 * C])
                nc.vector.tensor_copy(out=vp[:, hh, 0:C], in_=pv)

            # ---- attention, streamed in 512-column chunks ----
            y_tm = big.tile([P, T, C], F32, tag="y")
            for i in range(32):
                ech = ch.tile([128, 2, 512], BF16, tag="e")
                for hh in range(2):
                    pS = psA.tile([128, 512], F32, tag="ps", name="ps")
                    nc.tensor.matmul(out=pS,
                                     lhsT=kvcm[0:C, 128 * hh:128 * (hh + 1)],
                                     rhs=qt[:, 512 * i:512 * (i + 1)],
                                     start=True, stop=True)
                    nc.scalar.activation(out=ech[:, hh, :], in_=pS, func=AF.Exp)
                pO = psA.tile([128, 512], F32, tag="ps", name="ps")[0:C + 1, :]
                for hh in range(2):
                    nc.tensor.matmul(out=pO, lhsT=vp[:, hh, :],
                                     rhs=ech[:, hh, :],
                                     start=(hh == 0), stop=(hh == 1))
                pod = ch.tile([C + 1, 512], BF16, tag="pod")
                nc.scalar.copy(out=pod, in_=pO)
                pP = psA.tile([128, 512], F32, tag="ps", name="ps")[0:C + 1, :]
                nc.tensor.matmul(out=pP, lhsT=wpj, rhs=pod, start=True, stop=True)
                pd = ch.tile([C + 1, 512], BF16, tag="pd")
                nc.scalar.copy(out=pd, in_=pP)
                ptr = psT.tile([128, 4, C + 2], BF16, tag="tp")
                for k in range(4):
                    nc.tensor.transpose(out=ptr[:, k, 0:C + 1],
                                        in_=pd[:, 128 * k:128 * (k + 1)],
                                        identity=ident[0:C + 1, 0:C + 1])
                rt = sc.tile([P, 4, 1], F32, tag="rt")
                nc.vector.reciprocal(out=rt, in_=ptr[:, :, C:C + 1])
                tmp = ch.tile([P, 4, C], F32, tag="tmp")
                nc.vector.tensor_tensor(out=tmp, in0=ptr[:, :, 0:C],
                                        in1=rt.broadcast_to([P, 4, C]),
                                        op=OP.mult)
                nc.vector.tensor_tensor(out=y_tm[:, 4 * i:4 * (i + 1), :],
                                        in0=tmp, in1=x_tm[:, 4 * i:4 * (i + 1), :],
                                        op=OP.add)

            # ---- LN2 ----
            g2, mg2 = _ln_stats(nc, sc, y_tm,
                                sq_scr.rearrange("p (t c) -> p t c", c=C),
                                sq_scr, epst, T)
            a2tm = big.tile([P, T, C + 1], BF16, tag="scr2")
            nc.vector.tensor_tensor(
                out=a2tm[:, :, 0:C], in0=y_tm,
                in1=g2[:, :, None].broadcast_to([P, T, C]), op=OP.mult)
            nc.vector.tensor_copy(out=a2tm[:, :, C], in_=mg2)

            # ---- A2 guarded channel-major ----
            a2g = big.tile([C + 1, NG], BF16, tag="acm")
            nc.vector.memset(a2g[:, 0:PAD + RP], 0.0)
            nc.vector.memset(a2g[:, NG - PAD - RP:NG], 0.0)
            a2rows = a2g[:, PAD + RP:PAD + RP * (H + 1)].rearrange(
                "c (y w) -> c y w", w=RP)
            nc.vector.memset(a2rows[:, :, 0:1], 0.0)
            nc.vector.memset(a2rows[:, :, RP - 1:RP], 0.0)
            for j in range(32):
                pt = psT.tile([C + 1, 4, 128], BF16, tag="tp")
                for k in range(4):
                    nc.tensor.transpose(out=pt[:, k, :],
                                        in_=a2tm[:, 4 * j + k, :],
                                        identity=ident)
                nc.scalar.copy(out=a2rows[:, 4 * j:4 * (j + 1), 1:W + 1], in_=pt)

            # ---- MLP: fused fc1 (+) 3x3 depthwise conv, gelu, fc2 ----
            y2_tm = big.tile([P, T, C], F32, tag="xr")  # reuses x_tm slot
            out_v = out_d.rearrange("(t p) c -> p t c", p=P)
            for j in range(64):
                cb = PAD + RP * (2 * j + 1)
                gch = []
                for g in range(2):
                    pG = psA.tile([128, 512], F32, tag="ps", name="ps")[:, 0:2 * RP]
                    for tap in range(9):
                        dy, dx = tap // 3 - 1, tap % 3 - 1
                        off = RP * dy + dx
                        nc.tensor.matmul(out=pG, lhsT=wm[:, 2 * tap + g, :],
                                         rhs=a2g[:, cb + off:cb + off + 2 * RP],
                                         start=(tap == 0), stop=(tap == 8))
                    gc = ch.tile([128, 2 * RP], BF16, tag=f"gc{g}")
                    nc.scalar.activation(out=gc, in_=pG, func=AF.Gelu,
                                         bias=bg[:, g:g + 1], scale=1.0)
                    gch.append(gc)
                pF = psA.tile([128, 512], F32, tag="ps", name="ps")[0:C, 0:2 * RP]
                for g in range(2):
                    nc.tensor.matmul(out=pF, lhsT=wf2[:, g, :], rhs=gch[g],
                                     start=(g == 0), stop=(g == 1))
                o2 = ch.tile([C, 2 * RP], BF16, tag="o2")
                nc.scalar.activation(out=o2, in_=pF, func=AF.Identity,
                                     bias=bf2, scale=1.0)
                pt2 = psT.tile([128, 2, C], BF16, tag="tp")
                for k in range(2):
                    nc.tensor.transpose(out=pt2[:, k, :],
                                        in_=o2[:, RP * k + 1:RP * k + 1 + W],
                                        identity=ident[0:C, 0:C])
                nc.vector.tensor_tensor(out=y2_tm[:, 2 * j:2 * (j + 1), :],
                                        in0=pt2, in1=y_tm[:, 2 * j:2 * (j + 1), :],
                                        op=OP.add)
                if j % 8 == 7:
                    q8 = j // 8
                    nc.sync.dma_start(out=out_v[:, 16 * q8:16 * (q8 + 1), :],
                                      in_=y2_tm[:, 16 * q8:16 * (q8 + 1), :])

    _split_excess_waits(nc)
    return nc


@functools.cache
def _get_nc():
    return _build_nc()


def _prep_weights(inp):
    f = lambda v: np.asarray(v, np.float32)
    n1w, n1b = f(inp["n1_w"]), f(inp["n1_b"])
    q_w, q_b = f(inp["q_w"]), f(inp["q_b"])
    kv_w, kv_b = f(inp["kv_w"]), f(inp["kv_b"])
    sr_w, sr_b = f(inp["sr_w"]), f(inp["sr_b"])
    srnw, srnb = f(inp["srn_w"]), f(inp["srn_b"])
    pj_w, pj_b = f(inp["proj_w"]), f(inp["proj_b"])
    n2w, n2b = f(inp["n2_w"]), f(inp["n2_b"])
    f1w, f1b = f(inp["fc1_w"]), f(inp["fc1_b"])
    dww, dwb = f(inp["dw_w"]), f(inp["dw_b"])
    f2w, f2b = f(inp["fc2_w"]), f(inp["fc2_b"])

    scale = (C // 1) ** -0.5
    wq_l = np.concatenate([(q_w * n1w[None, :]).T, -(q_w @ n1w)[None, :]],
                          0) * scale
    bq_l = ((q_w @ n1b + q_b) * scale)[:, None]

    wsr_l = np.zeros((64, C + 1, C), np.float32)
    for kk in range(64):
        ky, kx = kk // 8, kk % 8
        wsr_l[kk, :C, :] = (sr_w[:, :, ky, kx] * n1w[None, :]).T
        wsr_l[kk, C, :] = -(sr_w[:, :, ky, kx] @ n1w)
    bsr_l = (sr_w.sum((2, 3)) @ n1b + sr_b)[:, None]

    wkv_l = (kv_w * srnw[None, :]).T
    bkv_l = (kv_w @ srnb + kv_b)[:, None]

    wpj_l = np.zeros((C + 1, C + 1), np.float32)
    wpj_l[:C, :C] = pj_w.T
    wpj_l[C, :C] = pj_b
    wpj_l[C, C] = 1.0

    k9 = dww[:, 0, :, :].reshape(HID, 9)          # [256, 9]
    wm_l = np.zeros((18, C + 1, 128), np.float32)
    for tap in range(9):
        M = k9[:, tap][:, None] * f1w             # [256, 64]
        Mw = M * n2w[None, :]
        for g in range(2):
            wm_l[2 * tap + g, :C, :] = Mw[128 * g:128 * (g + 1)].T
            wm_l[2 * tap + g, C, :] = -(M[128 * g:128 * (g + 1)] @ n2w)
    bg_full = k9.sum(1) * (f1w @ n2b + f1b) + dwb  # [256]
    bg_l = np.ascontiguousarray(bg_full.reshape(2, 128).T)

    wf2_l = np.stack([f2w[:, :128].T, f2w[:, 128:].T], 0)
    bf2_l = f2b[:, None]

    bfc = lambda a: np.ascontiguousarray(a).astype(BF)
    return {
        "wq": bfc(wq_l), "bq": np.ascontiguousarray(bq_l),
        "wsr": bfc(wsr_l), "bsr": np.ascontiguousarray(bsr_l),
        "wkv": bfc(wkv_l), "bkv": np.ascontiguousarray(bkv_l),
        "wpj": bfc(wpj_l),
        "wm": bfc(wm_l), "bg": np.ascontiguousarray(bg_l),
        "wf2": bfc(wf2_l), "bf2": np.ascontiguousarray(bf2_l),
    }


def kernel(trace=False, tmpdir=None, **inputs):
    nc = _get_nc()
    x = np.asarray(inputs["x"], np.float32)
    wts = _prep_weights(inputs)
    in_maps = [dict(wts, x=np.ascontiguousarray(x[b])) for b in range(B)]
    res = run_bass_kernel_spmd(nc, in_maps, core_ids=list(range(8)),
                               trace=trace, tmpdir=tmpdir)
    out = np.stack([res.results[b]["out"] for b in range(B)], 0)
    kernel.last_exec_time_ns = res.exec_time_ns
    return out


# revision 11
# speedup vs baseline: 1.1628x; 1.1628x over previous
"""Trainium2 Bass kernel for nn_Block_523986010339 (PVT-style transformer block).

Sharding: data-parallel over batch B=8 -> one batch element per NeuronCore.
Per-core layouts:
  - residual stream token-major fp32 [128p=token%128, 128t=token//128, 64c]
  - matmul operands channel-major bf16 [c, n], n = 128*y + x
  - LN mean folded into matmul weights via an extra "m*g" row; rsqrt scale
    applied token-major with broadcast APs
  - attention: S^T channel-major, exp without max-subtraction (tiny logits),
    denominator via fused ones-column in the V matmul, divided out after proj
  - MLP: fc1 and 3x3 depthwise conv fused into 9 accumulated matmuls over a
    zero-guarded channel-major layout (row pitch 130)
"""

import functools
import json

import numpy as np
import ml_dtypes

import concourse.bass as bass
import concourse.mybir as mybir
import concourse.tile as tile
from concourse.bass_utils import run_bass_kernel_spmd
from concourse.masks import make_identity

F32 = mybir.dt.float32
BF16 = mybir.dt.bfloat16
BF = ml_dtypes.bfloat16

B, N, C, H, W = 8, 16384, 64, 128, 128
SR, HID, NR = 8, 256, 256
P, T = 128, 128
RP = W + 2          # guarded row pitch
PAD = RP + 1        # head/tail pad so all tap offsets stay in-bounds
NG = PAD + RP * (H + 2) + PAD
AX = mybir.AxisListType
OP = mybir.AluOpType
AF = mybir.ActivationFunctionType


def _split_excess_waits(nc, max_waits=1):
    """walrus in this container rejects >1 sync wait per instruction; move
    excess waits onto injected Drain instructions just before the owner."""
    d = json.loads(mybir.module_to_json_string(nc.m))
    n_split = [0]

    def fix(insts):
        out = []
        for inst in insts:
            si = inst.get("sync_info") or {}
            waits = si.get("on_wait") or []
            if len(waits) > max_waits:
                extra = waits[:-max_waits]
                for i in range(0, len(extra), max_waits):
                    n_split[0] += 1
                    out.append({
                        "name": f"WSPLIT-{n_split[0]}",
                        "opcode": "Drain",
                        "engine": inst["engine"],
                        "ins": [],
                        "outs": [],
                        "is_reset_sema": False,
                        "sync_info": {"on_update": [],
                                      "on_wait": extra[i:i + max_waits]},
                    })
                si["on_wait"] = waits[-max_waits:]
                inst["sync_info"] = si
            out.append(inst)
        return out

    for f in d.get("functions", []):
        for bb in f.get("blocks", []):
            bb["instructions"] = fix(bb["instructions"])
    nc.m = mybir.module_from_json_string(json.dumps(d))


def _ln_stats(nc, sc, big, x_tm, epst, nt):
    """Token-major LN stats: returns (g, mg) tiles [128, nt] fp32 given
    x_tm [128, nt, 64] fp32."""
    sq_scr = big.tile([P, nt * C], BF16, tag="scr2", name="sq")
    xsq_view = sq_scr.rearrange("p (t c) -> p t c", c=C)
    nc.scalar.square(out=sq_scr, in_=x_tm.rearrange("p t c -> p (t c)"))
    s1 = sc.tile([P, nt], F32, tag=f"s1_{nt}")
    s2 = sc.tile([P, nt], F32, tag=f"s2_{nt}")
    nc.vector.tensor_reduce(out=s1, in_=x_tm, axis=AX.X, op=OP.add)
    nc.vector.tensor_reduce(out=s2, in_=xsq_view, axis=AX.X, op=OP.add)
    mean = sc.tile([P, nt], F32, tag=f"mean_{nt}")
    var = sc.tile([P, nt], F32, tag=f"var_{nt}")
    nc.vector.tensor_scalar_mul(out=mean, in0=s1, scalar1=1.0 / C)
    nc.vector.tensor_scalar_mul(out=var, in0=s2, scalar1=1.0 / C)
    mm = sc.tile([P, nt], F32, tag=f"mm_{nt}")
    nc.vector.tensor_tensor(out=mm, in0=mean, in1=mean, op=OP.mult)
    nc.vector.tensor_tensor(out=var, in0=var, in1=mm, op=OP.subtract)
    sd = sc.tile([P, nt], F32, tag=f"sd_{nt}")
    nc.scalar.activation(out=sd, in_=var, func=AF.Sqrt, bias=epst, scale=1.0)
    g = sc.tile([P, nt], F32, tag=f"g_{nt}")
    nc.vector.reciprocal(out=g, in_=sd)
    mg = sc.tile([P, nt], F32, tag=f"mg_{nt}")
    nc.vector.tensor_tensor(out=mg, in0=mean, in1=g, op=OP.mult)
    return g, mg


def _build_nc():
    nc = bass.Bass("TRN2")
    x_d = nc.dram_tensor("x", [N, C], F32, kind="ExternalInput")
    out_d = nc.dram_tensor("out", [N, C], F32, kind="ExternalOutput")
    wq_d = nc.dram_tensor("wq", [C + 1, C], BF16, kind="ExternalInput")
    bq_d = nc.dram_tensor("bq", [C, 1], F32, kind="ExternalInput")
    wsr_d = nc.dram_tensor("wsr", [64, C + 1, C], BF16, kind="ExternalInput")
    bsr_d = nc.dram_tensor("bsr", [C, 1], F32, kind="ExternalInput")
    wkv_d = nc.dram_tensor("wkv", [C, 2 * C], BF16, kind="ExternalInput")
    bkv_d = nc.dram_tensor("bkv", [2 * C, 1], F32, kind="ExternalInput")
    wpj_d = nc.dram_tensor("wpj", [C + 1, C + 1], BF16, kind="ExternalInput")
    wmp_d = nc.dram_tensor("wmp", [6, 128, 128], BF16, kind="ExternalInput")
    wms_d = nc.dram_tensor("wms", [6, C, 128], BF16, kind="ExternalInput")
    bg_d = nc.dram_tensor("bg", [128, 2], F32, kind="ExternalInput")
    wf2_d = nc.dram_tensor("wf2", [2, 128, C], BF16, kind="ExternalInput")
    bf2_d = nc.dram_tensor("bf2", [C, 1], F32, kind="ExternalInput")

    with tile.TileContext(nc) as tc:
        with (
            tc.tile_pool(name="consts", bufs=1) as consts,
            tc.tile_pool(name="big", bufs=1) as big,
            tc.tile_pool(name="sc", bufs=2) as sc,
            tc.tile_pool(name="ch", bufs=3) as ch,
            tc.tile_pool(name="psA", bufs=5, space="PSUM") as psA,
            tc.tile_pool(name="psT", bufs=2, space="PSUM") as psT,
        ):
            ident = consts.tile([128, 128], BF16)
            make_identity(nc, ident)
            wq = consts.tile([C + 1, C], BF16)
            nc.gpsimd.dma_start(out=wq, in_=wq_d[:, :])
            wsr = consts.tile([C + 1, 64, C], BF16)
            nc.gpsimd.dma_start(out=wsr, in_=wsr_d.rearrange("k c o -> c k o"))
            wkv = consts.tile([C, 2 * C], BF16)
            nc.gpsimd.dma_start(out=wkv, in_=wkv_d[:, :])
            wpj = consts.tile([C + 1, C + 1], BF16)
            nc.gpsimd.dma_start(out=wpj, in_=wpj_d[:, :])
            wmp = consts.tile([128, 6, 128], BF16)
            nc.gpsimd.dma_start(out=wmp, in_=wmp_d.rearrange("t c m -> c t m"))
            wms = consts.tile([C, 6, 128], BF16)
            nc.gpsimd.dma_start(out=wms, in_=wms_d.rearrange("t c m -> c t m"))
            wf2 = consts.tile([128, 2, C], BF16)
            nc.gpsimd.dma_start(out=wf2, in_=wf2_d.rearrange("g k m -> k g m"))
            bq = consts.tile([C, 1], F32)
            nc.gpsimd.dma_start(out=bq, in_=bq_d[:, :])
            bsr = consts.tile([C, 1], F32)
            nc.gpsimd.dma_start(out=bsr, in_=bsr_d[:, :])
            bkv = consts.tile([2 * C, 1], F32)
            nc.gpsimd.dma_start(out=bkv, in_=bkv_d[:, :])
            bg = consts.tile([128, 2], F32)
            nc.gpsimd.dma_start(out=bg, in_=bg_d[:, :])
            bf2 = consts.tile([C, 1], F32)
            nc.gpsimd.dma_start(out=bf2, in_=bf2_d[:, :])
            epst = consts.tile([P, 1], F32)
            nc.vector.memset(epst, 1e-5)

            # ---- load x (token-major) ----
            x_tm = big.tile([P, T, C], F32, tag="xr")
            x_v = x_d.rearrange("(t p) c -> p t c", p=P)
            for q4 in range(4):
                nc.sync.dma_start(out=x_tm[:, 32 * q4:32 * (q4 + 1), :],
                                  in_=x_v[:, 32 * q4:32 * (q4 + 1), :])

            # ---- LN1 ----
            g1, mg1 = _ln_stats(nc, sc, big, x_tm, epst, T)
            a1tm = big.tile([P, T, C + 1], BF16, tag="scr2")
            nc.vector.tensor_tensor(
                out=a1tm[:, :, 0:C], in0=x_tm,
                in1=g1[:, :, None].broadcast_to([P, T, C]), op=OP.mult)
            nc.vector.tensor_copy(out=a1tm[:, :, C], in_=mg1)

            # transpose A1 to channel-major [65, N]
            a1cm = big.tile([C + 1, N], BF16, tag="acm")
            a1cm_v = a1cm.rearrange("c (j k n) -> c j k n", j=32, k=4)
            for j in range(32):
                pt = psT.tile([C + 1, 4, 128], BF16, tag="tp")
                for k in range(4):
                    nc.tensor.transpose(out=pt[:, k, :],
                                        in_=a1tm[:, 4 * j + k, :],
                                        identity=ident)
                nc.scalar.copy(out=a1cm_v[:, j, :, :], in_=pt)

            # ---- Q^T = wq @ A1 ----
            qt = big.tile([C, N], BF16, tag="qt")
            for i in range(32):
                ps = psA.tile([128, 512], F32, tag="ps", name="ps")[0:C, :]
                nc.tensor.matmul(out=ps, lhsT=wq,
                                 rhs=a1cm[:, 512 * i:512 * (i + 1)],
                                 start=True, stop=True)
                nc.scalar.activation(out=qt[:, 512 * i:512 * (i + 1)], in_=ps,
                                     func=AF.Identity, bias=bq, scale=1.0)

            # ---- spatial reduction conv (8x8 stride 8) ----
            a1sr = a1cm.rearrange("c (Y ky X kx) -> c ky kx Y X", ky=SR, kx=SR, X=16)
            psr = psA.tile([128, 512], F32, tag="ps", name="ps").rearrange("c (a y x) -> c a y x", a=2, y=16)[0:C, 0, :, :]
            for kk in range(64):
                ky, kx = kk // 8, kk % 8
                nc.tensor.matmul(out=psr, lhsT=wsr[:, kk, :],
                                 rhs=a1sr[:, ky, kx, :, :],
                                 start=(kk == 0), stop=(kk == 63))
            xrcm = consts.tile([C, NR], BF16)
            nc.scalar.activation(out=xrcm.rearrange("c (y x) -> c y x", x=16),
                                 in_=psr, func=AF.Identity,
                                 bias=bsr, scale=1.0)

            # ---- LN on reduced tokens (srn), token-major ----
            xr_tm = consts.tile([P, 2, C], F32)
            for hh in range(2):
                pv = psT.tile([128, C], BF16, tag="tp")
                nc.tensor.transpose(out=pv, in_=xrcm[:, 128 * hh:128 * (hh + 1)],
                                    identity=ident[0:C, 0:C])
                nc.vector.tensor_copy(out=xr_tm[:, hh, :], in_=pv)
            g_r, mg_r = _ln_stats(nc, sc, consts, xr_tm, epst, 2)
            ar_tm = consts.tile([P, 2, C], BF16)
            nc.vector.tensor_tensor(
                out=ar_tm, in0=xr_tm,
                in1=g_r[:, :, None].broadcast_to([P, 2, C]), op=OP.mult)
            mgb = sc.tile([P, 2, C], BF16, tag="mgb")
            nc.vector.tensor_tensor(
                out=mgb, in0=mg_r[:, :, None].broadcast_to([P, 2, C]),
                in1=g_r[:, :, None].broadcast_to([P, 2, C]), op=OP.bypass)
            nc.vector.tensor_tensor(out=ar_tm, in0=ar_tm, in1=mgb, op=OP.subtract)
            arcm = consts.tile([C, NR], BF16)
            for hh in range(2):
                pv = psT.tile([C, 128], BF16, tag="tp")
                nc.tensor.transpose(out=pv, in_=ar_tm[:, hh, :], identity=ident)
                nc.vector.tensor_copy(out=arcm[:, 128 * hh:128 * (hh + 1)], in_=pv)

            # ---- KV ----
            pkv = psA.tile([128, 512], F32, tag="ps", name="ps")[:, 0:NR]
            nc.tensor.matmul(out=pkv, lhsT=wkv, rhs=arcm, start=True, stop=True)
            kvcm = consts.tile([2 * C, NR], BF16)
            nc.scalar.activation(out=kvcm, in_=pkv, func=AF.Identity,
                                 bias=bkv, scale=1.0)
            vp = consts.tile([128, 2, C + 1], BF16)
            nc.vector.memset(vp[:, :, C:C + 1], 1.0)
            for hh in range(2):
                pv = psT.tile([128, C], BF16, tag="tp")
                nc.tensor.transpose(out=pv,
                                    in_=kvcm[C:2 * C, 128 * hh:128 * (hh + 1)],
                                    identity=ident[C:2 * C, C:2 * C])
                nc.vector.tensor_copy(out=vp[:, hh, 0:C], in_=pv)

            # ---- attention, streamed in 512-column chunks ----
            y_tm = big.tile([P, T, C], F32, tag="y")
            for i in range(32):
                ech = ch.tile([128, 2, 512], BF16, tag="e")
                for hh in range(2):
                    pS = psA.tile([128, 512], F32, tag="ps", name="ps")
                    nc.tensor.matmul(out=pS,
                                     lhsT=kvcm[0:C, 128 * hh:128 * (hh + 1)],
                                     rhs=qt[:, 512 * i:512 * (i + 1)],
                                     start=True, stop=True)
                    nc.scalar.activation(out=ech[:, hh, :], in_=pS, func=AF.Exp)
                pO = psA.tile([128, 512], F32, tag="ps", name="ps")[0:C + 1, :]
                for hh in range(2):
                    nc.tensor.matmul(out=pO, lhsT=vp[:, hh, :],
                                     rhs=ech[:, hh, :],
                                     start=(hh == 0), stop=(hh == 1))
                pod = ch.tile([C + 1, 512], BF16, tag="pod")
                nc.vector.tensor_copy(out=pod, in_=pO)
                pP = psA.tile([128, 512], F32, tag="ps", name="ps")[0:C + 1, :]
                nc.tensor.matmul(out=pP, lhsT=wpj, rhs=pod, start=True, stop=True)
                pd = ch.tile([C + 1, 512], BF16, tag="pd")
                nc.scalar.copy(out=pd, in_=pP)
                ptr = psT.tile([128, 4, C + 2], BF16, tag="tp")
                for k in range(4):
                    nc.tensor.transpose(out=ptr[:, k, 0:C + 1],
                                        in_=pd[:, 128 * k:128 * (k + 1)],
                                        identity=ident[0:C + 1, 0:C + 1])
                rt = sc.tile([P, 4, 1], F32, tag="rt")
                nc.vector.reciprocal(out=rt, in_=ptr[:, :, C:C + 1])
                tmp = ch.tile([P, 4, C], F32, tag="tmp")
                nc.vector.tensor_tensor(out=tmp, in0=ptr[:, :, 0:C],
                                        in1=rt.broadcast_to([P, 4, C]),
                                        op=OP.mult)
                nc.vector.tensor_tensor(out=y_tm[:, 4 * i:4 * (i + 1), :],
                                        in0=tmp, in1=x_tm[:, 4 * i:4 * (i + 1), :],
                                        op=OP.add)

            # ---- LN2 ----
            g2, mg2 = _ln_stats(nc, sc, big, y_tm, epst, T)
            a2tm = big.tile([P, T, C + 1], BF16, tag="scr2")
            nc.vector.tensor_tensor(
                out=a2tm[:, :, 0:C], in0=y_tm,
                in1=g2[:, :, None].broadcast_to([P, T, C]), op=OP.mult)
            nc.vector.tensor_tensor(
                out=a2tm[:, :, 0:C], in0=a2tm[:, :, 0:C],
                in1=mg2[:, :, None].broadcast_to([P, T, C]), op=OP.subtract)

            # ---- A2 guarded channel-major, doubled: rows 64:128 shifted by +1 ----
            a2g = big.tile([128, NG], BF16, tag="acm")
            nc.vector.memset(a2g[:, 0:PAD + RP], 0.0)
            nc.vector.memset(a2g[:, NG - PAD - RP:NG], 0.0)
            a2rows = a2g[0:C, PAD + RP:PAD + RP * (H + 1)].rearrange(
                "c (y w) -> c y w", w=RP)
            a2rowsB = a2g[C:128, PAD + RP:PAD + RP * (H + 1)].rearrange(
                "c (y w) -> c y w", w=RP)
            nc.vector.memset(a2rows[:, :, 0:1], 0.0)
            nc.vector.memset(a2rows[:, :, RP - 1:RP], 0.0)
            nc.vector.memset(a2rowsB[:, :, RP - 2:RP], 0.0)
            for j in range(32):
                pt = psT.tile([C, 4, 128], BF16, tag="tp")
                for k in range(4):
                    nc.tensor.transpose(out=pt[:, k, :],
                                        in_=a2tm[:, 4 * j + k, 0:C],
                                        identity=ident)
                nc.scalar.copy(out=a2rows[:, 4 * j:4 * (j + 1), 1:W + 1], in_=pt)
                nc.vector.tensor_copy(out=a2rowsB[:, 4 * j:4 * (j + 1), 0:W],
                                      in_=pt)

            # ---- MLP: fused fc1 (+) 3x3 depthwise conv, gelu, fc2 ----
            o2cm = big.tile([C, NG], BF16, tag="qt")  # reuses qt slot
            n_mlp = 33
            for j in range(n_mlp):
                cb = PAD + RP + 512 * j
                size = min(512, PAD + RP * (H + 1) - cb)
                gch = []
                for g in range(2):
                    pG = psA.tile([128, 512], F32, tag="ps", name="ps")
                    for dy in (-1, 0, 1):
                        nc.tensor.matmul(
                            out=pG[:, 0:size], lhsT=wmp[:, 2 * (dy + 1) + g, :],
                            rhs=a2g[:, cb + RP * dy - 1:cb + RP * dy - 1 + size],
                            start=(dy == -1), stop=False)
                    for dy in (-1, 0, 1):
                        nc.tensor.matmul(
                            out=pG[:, 0:size], lhsT=wms[:, 2 * (dy + 1) + g, :],
                            rhs=a2g[0:C, cb + RP * dy + 1:cb + RP * dy + 1 + size],
                            start=False, stop=(dy == 1))
                    gc = ch.tile([128, 512], BF16, tag=f"gc{g}")
                    nc.scalar.activation(out=gc[:, 0:size], in_=pG[:, 0:size],
                                         func=AF.Gelu, bias=bg[:, g:g + 1],
                                         scale=1.0)
                    gch.append(gc)
                pF = psA.tile([128, 512], F32, tag="ps", name="ps")
                for g in range(2):
                    nc.tensor.matmul(out=pF[0:C, 0:size], lhsT=wf2[:, g, :],
                                     rhs=gch[g][:, 0:size],
                                     start=(g == 0), stop=(g == 1))
                nc.scalar.activation(out=o2cm[:, cb:cb + size],
                                     in_=pF[0:C, 0:size], func=AF.Identity,
                                     bias=bf2, scale=1.0)

            # ---- MLP epilogue: transpose back, residual, store ----
            y2_tm = big.tile([P, T, C], F32, tag="xr")  # reuses x_tm slot
            out_v = out_d.rearrange("(t p) c -> p t c", p=P)
            for j in range(32):
                pt2 = psT.tile([128, 4, C], BF16, tag="tp")
                for k in range(4):
                    t = 4 * j + k
                    s = PAD + RP * (t + 1) + 1
                    nc.tensor.transpose(out=pt2[:, k, :],
                                        in_=o2cm[:, s:s + W],
                                        identity=ident[0:C, 0:C])
                nc.vector.tensor_tensor(out=y2_tm[:, 4 * j:4 * (j + 1), :],
                                        in0=pt2, in1=y_tm[:, 4 * j:4 * (j + 1), :],
                                        op=OP.add)
                if j % 4 == 3:
                    q8 = j // 4
                    nc.sync.dma_start(out=out_v[:, 16 * q8:16 * (q8 + 1), :],
                                      in_=y2_tm[:, 16 * q8:16 * (q8 + 1), :])

    _split_excess_waits(nc)
    return nc


@functools.cache
def _get_nc():
    return _build_nc()


def _prep_weights(inp):
    f = lambda v: np.asarray(v, np.float32)
    n1w, n1b = f(inp["n1_w"]), f(inp["n1_b"])
    q_w, q_b = f(inp["q_w"]), f(inp["q_b"])
    kv_w, kv_b = f(inp["kv_w"]), f(inp["kv_b"])
    sr_w, sr_b = f(inp["sr_w"]), f(inp["sr_b"])
    srnw, srnb = f(inp["srn_w"]), f(inp["srn_b"])
    pj_w, pj_b = f(inp["proj_w"]), f(inp["proj_b"])
    n2w, n2b = f(inp["n2_w"]), f(inp["n2_b"])
    f1w, f1b = f(inp["fc1_w"]), f(inp["fc1_b"])
    dww, dwb = f(inp["dw_w"]), f(inp["dw_b"])
    f2w, f2b = f(inp["fc2_w"]), f(inp["fc2_b"])

    scale = (C // 1) ** -0.5
    wq_l = np.concatenate([(q_w * n1w[None, :]).T, -(q_w @ n1w)[None, :]],
                          0) * scale
    bq_l = ((q_w @ n1b + q_b) * scale)[:, None]

    wsr_l = np.zeros((64, C + 1, C), np.float32)
    for kk in range(64):
        ky, kx = kk // 8, kk % 8
        wsr_l[kk, :C, :] = (sr_w[:, :, ky, kx] * n1w[None, :]).T
        wsr_l[kk, C, :] = -(sr_w[:, :, ky, kx] @ n1w)
    bsr_l = (sr_w.sum((2, 3)) @ n1b + sr_b)[:, None]

    wkv_l = (kv_w * srnw[None, :]).T
    bkv_l = (kv_w @ srnb + kv_b)[:, None]

    wpj_l = np.zeros((C + 1, C + 1), np.float32)
    wpj_l[:C, :C] = pj_w.T
    wpj_l[C, :C] = pj_b
    wpj_l[C, C] = 1.0

    k9 = dww[:, 0, :, :].reshape(HID, 9)          # [256, 9]
    wmp_l = np.zeros((6, 128, 128), np.float32)
    wms_l = np.zeros((6, C, 128), np.float32)
    for dy in range(3):
        for g in range(2):
            Ma = (k9[:, dy * 3 + 0][:, None] * f1w * n2w[None, :])[128 * g:128 * (g + 1)]
            Mb = (k9[:, dy * 3 + 1][:, None] * f1w * n2w[None, :])[128 * g:128 * (g + 1)]
            Mc = (k9[:, dy * 3 + 2][:, None] * f1w * n2w[None, :])[128 * g:128 * (g + 1)]
            wmp_l[2 * dy + g, :C, :] = Ma.T
            wmp_l[2 * dy + g, C:, :] = Mb.T
            wms_l[2 * dy + g, :, :] = Mc.T
    bg_full = k9.sum(1) * (f1w @ n2b + f1b) + dwb  # [256]
    bg_l = np.ascontiguousarray(bg_full.reshape(2, 128).T)

    wf2_l = np.stack([f2w[:, :128].T, f2w[:, 128:].T], 0)
    bf2_l = f2b[:, None]

    bfc = lambda a: np.ascontiguousarray(a).astype(BF)
    return {
        "wq": bfc(wq_l), "bq": np.ascontiguousarray(bq_l),
        "wsr": bfc(wsr_l), "bsr": np.ascontiguousarray(bsr_l),
        "wkv": bfc(wkv_l), "bkv": np.ascontiguousarray(bkv_l),
        "wpj": bfc(wpj_l),
        "wmp": bfc(wmp_l), "wms": bfc(wms_l),
        "bg": np.ascontiguousarray(bg_l),
        "wf2": bfc(wf2_l), "bf2": np.ascontiguousarray(bf2_l),
    }


def kernel(trace=False, tmpdir=None, **inputs):
    nc = _get_nc()
    x = np.asarray(inputs["x"], np.float32)
    wts = _prep_weights(inputs)
    in_maps = [dict(wts, x=np.ascontiguousarray(x[b])) for b in range(B)]
    res = run_bass_kernel_spmd(nc, in_maps, core_ids=list(range(8)),
                               trace=trace, tmpdir=tmpdir)
    out = np.stack([res.results[b]["out"] for b in range(B)], 0)
    kernel.last_exec_time_ns = res.exec_time_ns
    return out
